# revision 15
# baseline (speedup 1.0000x reference)
"""Trainium2 Bass kernel for nn_AGCRNCellWithMLP (AGCRN cell with per-node MLP weights).

Math (with nodes_ind == arange(N), which the harness guarantees):
    xh       = concat([x, h], -1)                      # [N, 129]
    combined = adj @ xh                                # [N, 129]
    r = sigmoid(mlp(combined, q, W_r, b_r))            # [N, 64]
    u = sigmoid(mlp(combined, q, W_u, b_u))
    h2 = r * h
    cand = tanh(mlp(concat([x, h2], -1), q, W_c, b_c))
    out = (1 - u) * h2 + u * cand
where mlp(v, q, W, b)[n, o] = sum_{d,i} q[n,d] v[n,i] W[d,i,o] + (q @ b)[n, o].

Sharding: data-parallel over nodes, 512 rows per core x 8 cores, no
collectives. All matmul operands fp16 (rel err ~3e-3 vs 2e-2 gate), PSUM fp32,
output fp16.

Measured HW facts this kernel is shaped around:
  - PE cold (HAM-throttled) matmuls run at 1.2 GHz; any multi-us PE idle gap
    re-throttles. Dummy scrap matmuls warm it at start and across handoffs.
  - fp16 matmul with fresh 128-col weights: ~376ns at N=512 warm; 1-col-weight
    quad matmuls on disjoint col strips run concurrently.
  - DVE fp16 full-tile TT [128,1024]: ~680ns; broadcast APs (stride-0 dim) ok.
  - Each dma_start costs ~650ns on the Sync queue: batch DMAs aggressively.
  - ~7us fixed engine preamble; (1-u) comes free as sigmoid(-gru_u).

Per-core structure (full 512-node width, d-major gates):
  qrow    tiny first DMA: q rows flat + ones row -> PE broadcasts qbc_d
          [128,512] tiles (K=1 matmuls) interleaved through the adj phase.
  stream  one interleaved blob [xh-slabs | adjT-slabs] x 8 chunk-DMAs;
          combT = (adj @ xh)^T via 32 matmuls + 1-col quad matmuls for
          feature 128 (quad partials reduced by the sel17b matmul).
  gates   per qbc pair: one full-width DVE mul z2 = V2 (.) qbcpair (V2 read
          via stride-0 broadcast of combT / xh2T), two accumulating matmuls.
          Tail feature (i=128) via s-vectors and one K=16 matmul per gate
          group. Gate c feature order [h2|x(0:64)] puts its tail on the
          input-only x[:,64]; wdc blocks are duplicated to 128 cols so its
          matmuls hit the same fast path as ru.
  out     u' = sigmoid(-gru_u) early; out = u'*h2 + u*cand.
"""
import sys

sys.path.insert(0, "/opt/trn_rl_repo")

import numpy as np

N = 4096
IN = 64
QD = 16
CI = 2 * IN + 1          # 129
NCORES = 8
NS = N // NCORES         # 512 nodes per core
NS2 = 2 * NS             # paired width
KT = N // 128            # 32 k-tiles for the adj matmul
CI2 = CI + 1             # xh slab width: 129 + pad col
CW = CI2 + NS            # stream chunk width per k-tile (xh slab + adjT slab)

# blobA column offsets (all fp16, packed [128, BW])
_OFF = {}
_cols = 0
for _name, _w in [("qT", NS), ("bru", 2 * IN), ("bc", IN), ("sel17b", QD),
                  ("w128ru", 2 * IN), ("w128c", IN), ("x64rep", NS),
                  ("xTc", NS), ("hT", NS)]:
    _OFF[_name] = _cols
    _cols += _w
BW = _cols

_CACHE = {}


def build_nc():
    import concourse.bass as bass
    import concourse.bacc as bacc
    import concourse.tile as tile
    import concourse.mybir as mybir

    F32 = mybir.dt.float32
    F16 = mybir.dt.float16
    ACT = mybir.ActivationFunctionType

    nc = bacc.Bacc()
    dp = nc.declare_dram_parameter
    qrow_e = dp("qrow", [1, QD * NS + 128], F16, isOutput=False)  # q rows + ones
    blobA_e = dp("blobA", [128, BW], F16, isOutput=False)
    strm_e = dp("strm", [128, KT * CW], F16, isOutput=False)  # [xh_t | adjT_t] x32
    wdru_e = dp("wdru", [128, QD * 2 * IN], F16, isOutput=False)
    wdc_e = dp("wdc", [128, QD * 2 * IN], F16, isOutput=False)
    out_e = dp("out", [IN, NS], F16, isOutput=True)

    with tile.TileContext(nc) as tc:
        with tc.tile_pool(name="const", bufs=1) as cpool, \
             tc.tile_pool(name="big", bufs=1) as bigpool, \
             tc.tile_pool(name="work", bufs=1) as wpool, \
             tc.tile_pool(name="zt", bufs=3) as ztpool, \
             tc.tile_pool(name="psM", bufs=1, space="PSUM") as psM, \
             tc.tile_pool(name="psQ", bufs=2, space="PSUM") as psQ:

            # ---- DMAs: qrow first, 2 stream chunks, blobA, rest ------------
            qrow = cpool.tile([1, QD * NS + 128], F16, tag="qrow")
            nc.sync.dma_start(qrow[:], qrow_e[:])
            strm = bigpool.tile([128, KT * CW], F16)

            def strm_chunk(g):
                lo, hi = g * 4 * CW, (g + 1) * 4 * CW
                nc.sync.dma_start(strm[:, lo:hi], strm_e[:, lo:hi])

            strm_chunk(0)
            strm_chunk(1)
            blobA = cpool.tile([128, BW], F16, tag="blobA")
            nc.sync.dma_start(blobA[:], blobA_e[:])
            for g in range(2, 8):
                strm_chunk(g)
            wdru = cpool.tile([128, QD * 2 * IN], F16, tag="wdru")
            nc.sync.dma_start(wdru[:], wdru_e[:])
            wdc = cpool.tile([128, QD * 2 * IN], F16, tag="wdc")
            nc.sync.dma_start(wdc[:], wdc_e[:])

            def cslice(name, w, p=128):
                o = _OFF[name]
                return blobA[0:p, o:o + w]

            qT = cslice("qT", NS, QD)
            bru = cslice("bru", 2 * IN, QD)
            bc = cslice("bc", IN, QD)
            sel17b = cslice("sel17b", QD, 128)
            w128ru = cslice("w128ru", 2 * IN, QD)
            w128c = cslice("w128c", IN, QD)
            x64rep = cslice("x64rep", NS, QD)
            xTc = cslice("xTc", NS, IN)
            hT = cslice("hT", NS, IN)
            ones1 = qrow[0:1, QD * NS:QD * NS + 128]

            def xh_t(t, a, b):
                return strm[:, t * CW + a: t * CW + b]

            def adj_t(t):
                return strm[:, t * CW + CI2:(t + 1) * CW]

            # ---- warmup: dummy matmuls on scrap keep/get the PE hot --------
            scrap = cpool.tile([128, NS], F16, tag="scrap")
            nc.vector.memset(scrap[:], 0.0)
            ps_scrap = psM.tile([QD, NS], F32, tag="scrapps")

            def warm(n, name):
                for w in range(n):
                    nc.tensor.matmul(ps_scrap[:], scrap[0:QD, 0:QD],
                                     scrap[0:QD, :], start=True, stop=True,
                                     skip_group_check=True)

            warm(8, "w0")

            # qbc_d [128, NS] broadcasts, stored as pair tiles [128, NS2]
            qbp = [cpool.tile([128, NS2], F16, tag=f"qbp{j}", name=f"qbp{j}")
                   for j in range(8)]

            def qbc_gen(d):
                ps = psQ.tile([128, NS], F32, tag="qb", name=f"qb{d}")
                nc.tensor.matmul(ps[:], ones1,
                                 qrow[0:1, d * NS:(d + 1) * NS],
                                 start=True, stop=True)
                dst = qbp[d // 2][:, (d % 2) * NS:(d % 2 + 1) * NS]
                if d % 2 == 0:
                    nc.scalar.copy(dst, ps[:])
                else:
                    nc.vector.tensor_copy(dst, ps[:])

            for d in range(4):
                qbc_gen(d)

            # preload sigmoid/tanh activation tables off the critical path
            scr_act = wpool.tile([1, 2], F16, tag="scr_act")
            nc.scalar.activation(scr_act[:], scrap[0:1, 0:2], ACT.Sigmoid)
            scr_act2 = wpool.tile([1, 2], F16, tag="scr_act2")
            nc.scalar.activation(scr_act2[:], scrap[0:1, 0:2], ACT.Tanh)

            # xh2T = [h2|x]: x copied into rows 64:128 on ScalarE
            xh2T = wpool.tile([128, NS], F16, tag="xh2T")
            nc.scalar.copy(xh2T[64:128, :], xTc)
            # s_c = q (.) x64, needs only inputs
            s_c = wpool.tile([QD, NS], F16, tag="s_c")
            nc.vector.tensor_mul(s_c[:], qT, x64rep)

            # ---- gate bias matmuls open the PSUM accumulations -------------
            gru = psM.tile([2 * IN, NS], F32, tag="gru")
            gc2 = psM.tile([128, NS], F32, tag="gc")
            gc = gc2[0:IN, :]
            nc.tensor.matmul(gru[:], bru, qT,
                             start=True, stop=False, skip_group_check=True)
            nc.tensor.matmul(gc, bc, qT,
                             start=True, stop=False, skip_group_check=True)

            # ---- adj matmul: pc + pl quads + interleaved qbc gen -----------
            pc = psM.tile([128, NS], F32, tag="pc")
            pl = psM.tile([128, NS], F32, tag="pl")
            for g in range(8):
                for t in range(4 * g, 4 * g + 4):
                    nc.tensor.matmul(pc[:], xh_t(t, 0, 128), adj_t(t),
                                     start=(t == 0), stop=(t == KT - 1),
                                     skip_group_check=True)
                for t in range(4 * g, 4 * g + 4):
                    j = t % 4
                    nc.tensor.matmul(pl[32 * j:32 * j + 1, :],
                                     xh_t(t, 128, 129), adj_t(t),
                                     start=(g == 0), stop=(g == 7),
                                     tile_position=(0, 32 * j),
                                     skip_group_check=True)
                if g < 6:
                    qbc_gen(4 + 2 * g)
                    qbc_gen(5 + 2 * g)
            combT = wpool.tile([128, NS], F16, tag="combT")
            nc.scalar.copy(combT[:], pc[:])
            combT2 = combT[:].unsqueeze(1).broadcast_to([128, 2, NS])
            warm(3, "w1")

            # ---- gates r, u (d-major, paired z) ----------------------------
            def z2_mul(j, V2, name):
                z2 = ztpool.tile([128, NS2], F16, tag="z", name=name)
                z2v = z2[:].rearrange("p (two n) -> p two n", two=2)
                nc.vector.tensor_mul(
                    z2v, V2,
                    qbp[j][:].rearrange("p (two n) -> p two n", two=2))
                return z2

            def wf_pair(j, z2, ps, w):
                for k in range(2):
                    d = 2 * j + k
                    c0 = d * 2 * IN
                    nc.tensor.matmul(ps, w[:, c0:c0 + 2 * IN],
                                     z2[:, k * NS:(k + 1) * NS],
                                     start=False, stop=False,
                                     skip_group_check=True)

            for j in range(8):
                z2 = z2_mul(j, combT2, f"zru{j}")
                wf_pair(j, z2, gru[:], wdru)
                if j == 0:
                    # tail inputs, off the critical path of the first pairs
                    pl_sb = wpool.tile([128, NS], F16, tag="pl_sb")
                    nc.scalar.copy(pl_sb[:], pl[:])
            v128 = psQ.tile([QD, NS], F32, tag="qb", name="v128")
            nc.tensor.matmul(v128[:], sel17b, pl_sb[:], start=True, stop=True)
            s_ru = wpool.tile([QD, NS], F16, tag="s_ru")
            nc.vector.tensor_mul(s_ru[:], qT, v128[:])
            nc.tensor.matmul(gru[:], w128ru, s_ru[:],
                             start=False, stop=True, skip_group_check=True)
            warm(4, "w2")
            r_sb = wpool.tile([IN, NS], F16, tag="r_sb")
            nc.scalar.activation(r_sb[:], gru[0:IN, :], ACT.Sigmoid)

            # h2 = r*h -> xh2T rows 0:64
            nc.vector.tensor_mul(xh2T[0:IN, :], r_sb[:], hT)
            xh2T2 = xh2T[:].unsqueeze(1).broadcast_to([128, 2, NS])

            # ---- gate c (d-major over [h2 | x], paired z) ------------------
            for j in range(8):
                z2 = z2_mul(j, xh2T2, f"zc{j}")
                wf_pair(j, z2, gc2[:], wdc)
                if j == 0:
                    # u and u' = 1-u on ScalarE while DVE/PE run the c loop
                    u_sb = wpool.tile([IN, NS], F16, tag="u_sb")
                    nc.scalar.activation(u_sb[:], gru[IN:2 * IN, :], ACT.Sigmoid)
                    up_sb = wpool.tile([IN, NS], F16, tag="up_sb")
                    nc.scalar.activation(up_sb[:], gru[IN:2 * IN, :],
                                         ACT.Sigmoid, scale=-1.0)
                if j == 1:
                    # e2 = (1-u)*h2, also early
                    e2 = wpool.tile([IN, NS], F16, tag="e2")
                    nc.vector.tensor_mul(e2[:], up_sb[:], xh2T[0:IN, :])
            nc.tensor.matmul(gc, w128c, s_c[:],
                             start=False, stop=True, skip_group_check=True)
            cand = wpool.tile([IN, NS], F16, tag="cand")
            nc.scalar.activation(cand[:], gc, ACT.Tanh)

            # ---- out = u*cand + (1-u)*h2 -----------------------------------
            e1 = wpool.tile([IN, NS], F16, tag="e1")
            nc.vector.tensor_mul(e1[:], u_sb[:], cand[:])
            outT = wpool.tile([IN, NS], F16, tag="outT")
            nc.vector.tensor_add(outT[:], e1[:], e2[:])
            nc.sync.dma_start(out_e[:], outT[:])
    nc.compile()
    return nc


def _f16(a):
    return np.ascontiguousarray(np.asarray(a, np.float16))


def prep_in_maps(x, h, query_vectors, adj, nodes_ind, W_r, b_r, W_u, b_u, W_c, b_c):
    x = np.asarray(x, np.float32)
    h = np.asarray(h, np.float32)
    q = np.asarray(query_vectors, np.float32)
    adj = np.asarray(adj, np.float32)
    ni = np.asarray(nodes_ind)
    assert np.array_equal(ni, np.arange(N)), "kernel assumes nodes_ind == arange(N)"

    xh = np.concatenate([x, h, np.zeros((N, 1), np.float32)], axis=-1)  # [N,130]
    xh_kt = xh.reshape(KT, 128, CI2).transpose(1, 0, 2)     # [128, KT, 130]

    Wr = np.asarray(W_r, np.float32)
    Wu = np.asarray(W_u, np.float32)
    Wc = np.asarray(W_c, np.float32)
    wdru = np.concatenate([Wr[:, :128, :], Wu[:, :128, :]], axis=2)  # [16,128,128]
    wdru = _f16(wdru.transpose(1, 0, 2).reshape(128, QD * 2 * IN))
    perm_c = list(range(65, CI)) + list(range(0, 64))                # [h2|x]
    wdc1 = Wc[:, perm_c, :]                                          # [16,128,64]
    wdc = _f16(np.concatenate([wdc1, wdc1], axis=2)
               .transpose(1, 0, 2).reshape(128, QD * 2 * IN))

    sel17b = np.zeros((128, QD), np.float32)
    for jj in range(4):
        sel17b[32 * jj, :] = 1.0

    in_maps = []
    for c in range(NCORES):
        s = slice(c * NS, (c + 1) * NS)
        qs = q[s].T                                             # [16, 512]

        blobA = np.zeros((128, BW), np.float32)

        def put(name, arr):
            o = _OFF[name]
            blobA[0:arr.shape[0], o:o + arr.shape[1]] = arr

        put("qT", qs)
        put("bru", np.concatenate([np.asarray(b_r, np.float32),
                                   np.asarray(b_u, np.float32)], axis=1))
        put("bc", np.asarray(b_c, np.float32))
        put("sel17b", sel17b)
        put("w128ru", np.concatenate([Wr[:, 128, :], Wu[:, 128, :]], axis=1))
        put("w128c", Wc[:, 64, :])
        put("x64rep", np.tile(x[s, 64], (QD, 1)))
        put("xTc", x[s, 0:64].T)
        put("hT", h[s].T)

        adjT_kt = adj[s].T.reshape(KT, 128, NS).transpose(1, 0, 2)  # [128,KT,NS]
        strm = np.concatenate([xh_kt, adjT_kt], axis=2)             # [128,KT,CW]
        strm = _f16(strm.reshape(128, KT * CW))

        qrow = np.concatenate([qs.reshape(1, QD * NS),
                               np.ones((1, 128), np.float32)], axis=1)

        in_maps.append({
            "qrow": _f16(qrow),
            "blobA": _f16(blobA),
            "strm": strm,
            "wdru": wdru, "wdc": wdc,
        })
    return in_maps


def kernel(**inputs):
    from concourse.bass_utils import run_bass_kernel_spmd

    if "nc" not in _CACHE:
        _CACHE["nc"] = build_nc()
    nc = _CACHE["nc"]
    in_maps = prep_in_maps(**inputs)
    res = run_bass_kernel_spmd(nc, in_maps, core_ids=list(range(NCORES)))
    out = np.empty((N, IN), np.float32)
    for c in range(NCORES):
        out[c * NS:(c + 1) * NS, :] = res.results[c]["out"].T.astype(np.float32)
    return out
